# revision 1
# baseline (speedup 1.0000x reference)
"""Trainium2 Bass kernel for the MixedGNN problem (GCN -> GAT -> SAGE -> linear+log_softmax).

Sharding: nodes are permuted into 128-node blocks balanced by in-degree; each of the
8 cores owns a contiguous range of blocks (its slab). Edges live with their
destination block (self loops are explicit edges for GCN/GAT, disabled for SAGE).
Per-edge source rows are fetched with dma_gather (int16 indices, so gather tables
are split into two halves and each block's edges are grouped by source half).
Segment sums are one-hot matmuls accumulating in PSUM; GCN normalization and the
GAT softmax are applied per node, not per edge. Node tables needed by later layers
are exchanged with AllGather.

Host-side work is integer-only packing/permutation metadata; all floating-point
model math runs on the NeuronCores.
"""

import os
import sys
import heapq

import numpy as np

sys.path.insert(0, "/opt/trn_rl_repo")

import concourse.tile as tile  # noqa: E402
from concourse import bacc, mybir  # noqa: E402
from concourse.bass_utils import run_bass_kernel_spmd  # noqa: E402

F32 = mybir.dt.float32
BF16 = mybir.dt.bfloat16
I16 = mybir.dt.int16
ALU = mybir.AluOpType
ACTF = mybir.ActivationFunctionType

NC = 8
P = 128
D_IN = 128
D_H = 128
H = 2
D_OUT = 32
NEG_SLOPE = 0.2
TBLW = 320  # hw-table row stride in f32 (256 hw | 2 a_s | pad) -- 1280B, %256==0


# ----------------------------------------------------------------------------
# Host packing (integer only)
# ----------------------------------------------------------------------------

def _assign_blocks(w, nblk, rng):
    """Greedy balanced assignment of nodes to blocks (<=128 nodes each)."""
    n = len(w)
    order = np.lexsort((rng.permutation(n), -w))
    blk_of = np.empty(n, np.int32)
    heap = [(0, b) for b in range(nblk)]
    heapq.heapify(heap)
    nodecnt = np.zeros(nblk, np.int32)
    for i in order:
        load, b = heapq.heappop(heap)
        blk_of[i] = b
        nodecnt[b] += 1
        if nodecnt[b] < P:
            heapq.heappush(heap, (load + int(w[i]), b))
    return blk_of


def _pack(edge_index, N):
    E = edge_index.shape[1]
    src = np.asarray(edge_index[0], dtype=np.int64)
    dst = np.asarray(edge_index[1], dtype=np.int64)
    NBLK = NC * int(np.ceil(N / (P * NC)))
    NPAD = NBLK * P
    HALF = NPAD // 2
    BPC = NBLK // NC
    SLAB = BPC * P

    deg_in = np.bincount(dst, minlength=N).astype(np.int64)
    w = deg_in + 1  # incoming edges incl. self loop

    best = None
    rng = np.random.default_rng(1234)
    for _try in range(6):
        blk_of = _assign_blocks(w, NBLK, rng)
        order = np.argsort(blk_of, kind="stable")
        cnt = np.bincount(blk_of, minlength=NBLK)
        starts = np.zeros(NBLK + 1, np.int64)
        np.cumsum(cnt, out=starts[1:])
        slot = np.arange(N) - starts[blk_of[order]]
        perm = np.empty(N, np.int64)
        perm[order] = blk_of[order] * P + slot
        esrc = np.concatenate([src, np.arange(N)])
        edst = np.concatenate([dst, np.arange(N)])
        psrc = perm[esrc]
        pdst = perm[edst]
        key = (pdst >> 7) * 2 + (psrc >= HALF)
        counts = np.bincount(key, minlength=NBLK * 2)
        t_half = int(np.ceil(counts.max() / P))
        if best is None or t_half < best[0]:
            best = (t_half, perm, psrc, pdst, counts)
        if t_half <= max(2, int(np.ceil(counts.mean() / P))):
            break
    t_half, perm, psrc, pdst, counts = best
    T = 2 * t_half
    SLOT = t_half * P

    esrc = np.concatenate([src, np.arange(N)])
    is_self = np.concatenate([np.zeros(E, bool), np.ones(N, bool)])
    key = (pdst >> 7) * 2 + (psrc >= HALF)
    ordr = np.lexsort((psrc, key))
    ks = key[ordr]
    grp_start = np.concatenate(([0], np.cumsum(counts)))[ks]
    pos_in_grp = np.arange(len(ks)) - grp_start
    slot_pos = ks * SLOT + pos_in_grp

    tot = NBLK * 2 * SLOT
    eidx = np.zeros(tot, np.int64)
    edl = np.full(tot, -1.0, np.float32)
    edeg = np.ones(tot, np.float32)
    esg = np.full(tot, -1.0, np.float32)
    eidx[slot_pos] = psrc[ordr] - (ks % 2) * HALF
    edl[slot_pos] = (pdst[ordr] & 127).astype(np.float32)
    edeg[slot_pos] = w[esrc[ordr]].astype(np.float32)
    esg[slot_pos] = np.where(is_self[ordr], -1.0, (pdst[ordr] & 127).astype(np.float32))

    assert eidx.max() < HALF and eidx.min() >= 0
    eidx16 = eidx.astype(np.int16)

    # idx tiles: flat i -> [i%16, i//16], replicated x8 down partitions
    A = eidx16.reshape(NBLK, 2, SLOT // 16, 16).transpose(0, 1, 3, 2)
    idx_full = np.ascontiguousarray(np.tile(A, (1, 1, 8, 1)))

    edl_r = edl.reshape(NBLK, T, P).transpose(0, 2, 1)
    edeg_r = edeg.reshape(NBLK, T, P).transpose(0, 2, 1)
    esg_r = esg.reshape(NBLK, T, P).transpose(0, 2, 1)
    meta = np.ascontiguousarray(
        np.concatenate([edl_r, edeg_r, esg_r], axis=2).astype(np.float32))
    metaT = np.ascontiguousarray(edl.reshape(NBLK, T * P).astype(np.float32))

    w_p = np.ones(NPAD, np.float32)
    w_p[perm] = w.astype(np.float32)
    sg_p = np.ones(NPAD, np.float32)
    sg_p[perm] = np.maximum(deg_in, 1).astype(np.float32)
    degs = np.ascontiguousarray(
        np.stack([w_p.reshape(NBLK, P), sg_p.reshape(NBLK, P)], axis=2))

    return dict(
        NBLK=NBLK, NPAD=NPAD, HALF=HALF, BPC=BPC, SLAB=SLAB,
        T_half=t_half, T=T, perm=perm,
        idx=idx_full, meta=meta, metaT=metaT, degs=degs,
    )


# ----------------------------------------------------------------------------
# Device program
# ----------------------------------------------------------------------------

def _build_program(pk):
    BPC, T, Th, NPAD, HALF, SLAB = (
        pk["BPC"], pk["T"], pk["T_half"], pk["NPAD"], pk["HALF"], pk["SLAB"])
    NI = Th * P  # idxs per gather

    nc = bacc.Bacc("TRN2", target_bir_lowering=False, num_devices=NC,
                   num_swdge_queues=4, dynamic_dma_scratch_size=65536)

    x_perm = nc.dram_tensor("x_perm", [NPAD, D_IN], F32, kind="ExternalInput")
    idx_d = nc.dram_tensor("idx", [BPC, 2, P, NI // 16], I16, kind="ExternalInput")
    meta_d = nc.dram_tensor("meta", [BPC, P, 3 * T], F32, kind="ExternalInput")
    metaT_d = nc.dram_tensor("metaT", [BPC, T * P], F32, kind="ExternalInput")
    degs_d = nc.dram_tensor("degs", [BPC, P, 2], F32, kind="ExternalInput")
    w_gcn_d = nc.dram_tensor("w_gcn", [D_IN, D_H], F32, kind="ExternalInput")
    w_gat_d = nc.dram_tensor("w_gat", [D_H, H * D_H], F32, kind="ExternalInput")
    att_s_d = nc.dram_tensor("att_s", [P, H * D_H], F32, kind="ExternalInput")
    att_d_d = nc.dram_tensor("att_d", [P, H * D_H], F32, kind="ExternalInput")
    w_sl_d = nc.dram_tensor("w_sl", [D_H, D_H], F32, kind="ExternalInput")
    w_sr_d = nc.dram_tensor("w_sr", [D_H, D_H], F32, kind="ExternalInput")
    w_out_d = nc.dram_tensor("w_out", [D_H, D_OUT], F32, kind="ExternalInput")
    ident_d = nc.dram_tensor("ident", [P, P], F32, kind="ExternalInput")
    iotar_d = nc.dram_tensor("iotar", [P, P], F32, kind="ExternalInput")
    iotac_d = nc.dram_tensor("iotac", [P, 1], F32, kind="ExternalInput")
    onesr_d = nc.dram_tensor("onesr", [1, P], F32, kind="ExternalInput")
    out_d = nc.dram_tensor("out", [SLAB, D_OUT], F32, kind="ExternalOutput")

    rg = [list(range(NC))]
    qn = [0]

    def next_q():
        qn[0] = (qn[0] + 1) % 4
        return qn[0]

    with tile.TileContext(nc) as tc:
        with (
            tc.tile_pool(name="const", bufs=1) as cp,
            tc.tile_pool(name="dram", bufs=1, space="DRAM") as dp,
        ):
            def cload(shape, dt, src, tag):
                t = cp.tile(shape, dt, tag=tag)
                nc.sync.dma_start(out=t[:], in_=src)
                return t

            w_gcn = cload([D_IN, D_H], F32, w_gcn_d[:], "c_wgcn")
            w_gat = cload([D_H, H * D_H], F32, w_gat_d[:], "c_wgat")
            att_s = cload([P, H * D_H], F32, att_s_d[:], "c_atts")
            att_dt = cload([P, H * D_H], F32, att_d_d[:], "c_attd")
            w_sl = cload([D_H, D_H], F32, w_sl_d[:], "c_wsl")
            w_sr = cload([D_H, D_H], F32, w_sr_d[:], "c_wsr")
            w_out = cload([D_H, D_OUT], F32, w_out_d[:], "c_wout")
            ident = cload([P, P], F32, ident_d[:], "c_ident")
            iotar = cload([P, P], F32, iotar_d[:], "c_iotar")
            iotac = cload([P, 1], F32, iotac_d[:], "c_iotac")
            onesr = cload([1, P], F32, onesr_d[:], "c_onesr")

            meta_res = cp.tile([P, BPC * 3 * T], F32)
            for b in range(BPC):
                nc.sync.dma_start(out=meta_res[:, b * 3 * T:(b + 1) * 3 * T],
                                  in_=meta_d[b])
            degs_res = cp.tile([P, BPC * 2], F32)
            for b in range(BPC):
                nc.sync.dma_start(out=degs_res[:, b * 2:(b + 1) * 2], in_=degs_d[b])

            h1_sb = cp.tile([P, SLAB], F32)   # GCN output slab (reused for h3)
            h2_sb = cp.tile([P, SLAB], F32)   # GAT output slab
            ad_sb = cp.tile([P, 2 * BPC], BF16)  # per-own-node a_d

            hwt_slab = dp.tile([SLAB, TBLW], F32)
            hwt_full = dp.tile([NPAD, TBLW], F32)
            h2_slab = dp.tile([SLAB, D_H], F32)
            h2_full = dp.tile([NPAD, D_H], F32)

            def mcol(b, t):
                return meta_res[:, b * 3 * T + t:b * 3 * T + t + 1]

            def mdeg_cols(b):
                return meta_res[:, b * 3 * T + T:b * 3 * T + 2 * T]

            def msage(b, t):
                return meta_res[:, b * 3 * T + 2 * T + t:b * 3 * T + 2 * T + t + 1]

            # =============== Layer 1: GCN (+ hw table build) ===============
            with (
                tc.tile_pool(name="l1w", bufs=3) as wp,
                tc.tile_pool(name="l1p", bufs=2, space="PSUM") as pp,
                tc.tile_pool(name="l1pt", bufs=1, space="PSUM") as ppt,
                tc.tile_pool(name="l1ph", bufs=1, space="PSUM") as pph,
            ):
                for b in range(BPC):
                    g0 = wp.tile([P, Th * D_IN], F32, tag="g1a")
                    g1 = wp.tile([P, Th * D_IN], F32, tag="g1b")
                    for h, g in ((0, g0), (1, g1)):
                        src_ap = x_perm[:] if h == 0 else x_perm[HALF:, :]
                        ixt = wp.tile([P, NI // 16], I16, tag="ix1")
                        nc.sync.dma_start(out=ixt[:], in_=idx_d[b, h])
                        nc.gpsimd.dma_gather(
                            out_ap=g[:].rearrange("p (t w) -> p t w", w=D_IN),
                            in_ap=src_ap,
                            idxs_ap=ixt[:],
                            num_idxs=NI, num_idxs_reg=NI, elem_size=D_IN,
                            single_packet=False, queue_num=next_q())
                    dinv = wp.tile([P, T], F32, tag="dinv")
                    nc.scalar.activation(out=dinv[:], in_=mdeg_cols(b), func=ACTF.Sqrt)
                    nc.vector.reciprocal(out=dinv[:], in_=dinv[:])
                    psum = pp.tile([P, D_H], F32, tag="pg")
                    for t in range(T):
                        h, tr = divmod(t, Th)
                        g = g0 if h == 0 else g1
                        oh = wp.tile([P, P], F32, tag="oh1")
                        nc.vector.tensor_scalar(
                            out=oh[:], in0=iotar[:], scalar1=mcol(b, t),
                            scalar2=dinv[:, t:t + 1], op0=ALU.is_equal, op1=ALU.mult)
                        nc.tensor.matmul(
                            out=psum[:], lhsT=oh[:],
                            rhs=g[:, tr * D_IN:(tr + 1) * D_IN],
                            start=(t == 0), stop=(t == T - 1))
                    # epilogue: h1 = relu((dinv_i * psum) @ W_gcn)
                    dinv_o = wp.tile([P, 1], F32, tag="dv0")
                    nc.scalar.activation(out=dinv_o[:],
                                         in_=degs_res[:, 2 * b:2 * b + 1],
                                         func=ACTF.Sqrt)
                    nc.vector.reciprocal(out=dinv_o[:], in_=dinv_o[:])
                    pre = wp.tile([P, D_IN], F32, tag="pre")
                    nc.vector.tensor_scalar(out=pre[:], in0=psum[:], scalar1=dinv_o[:],
                                            scalar2=None, op0=ALU.mult)
                    tps0 = ppt.tile([P, P], F32, tag="tr1")
                    nc.tensor.transpose(out=tps0[:], in_=pre[:], identity=ident[:])
                    preT = wp.tile([P, P], F32, tag="preT")
                    nc.vector.tensor_copy(out=preT[:], in_=tps0[:])
                    gcn_ps = pph.tile([P, D_H], F32, tag="gc")
                    nc.tensor.matmul(out=gcn_ps[:], lhsT=preT[:], rhs=w_gcn[:],
                                     start=True, stop=True)
                    h1_blk = h1_sb[:, b * P:(b + 1) * P]
                    nc.scalar.activation(out=h1_blk, in_=gcn_ps[:], func=ACTF.Relu)
                    # hw table build
                    tps = ppt.tile([P, P], F32, tag="tr1")
                    nc.tensor.transpose(out=tps[:], in_=h1_blk, identity=ident[:])
                    h1T = wp.tile([P, P], F32, tag="h1T")
                    nc.vector.tensor_copy(out=h1T[:], in_=tps[:])
                    hw_ps = pph.tile([P, H * D_H], F32, tag="hw")
                    nc.tensor.matmul(out=hw_ps[:], lhsT=h1T[:], rhs=w_gat[:],
                                     start=True, stop=True)
                    tw = wp.tile([P, TBLW], F32, tag="tw")
                    nc.vector.tensor_copy(out=tw[:, 0:H * D_H], in_=hw_ps[:])
                    scr = wp.tile([P, H * D_H], F32, tag="scr")
                    nc.vector.tensor_tensor(out=scr[:], in0=tw[:, 0:H * D_H],
                                            in1=att_s[:], op=ALU.mult)
                    nc.vector.reduce_sum(
                        out=tw[:, 256:258].rearrange("p (a b) -> p a b", b=1),
                        in_=scr[:].rearrange("p (a c) -> p a c", c=D_H),
                        axis=mybir.AxisListType.X)
                    nc.vector.tensor_tensor(out=scr[:], in0=tw[:, 0:H * D_H],
                                            in1=att_dt[:], op=ALU.mult)
                    with nc.allow_low_precision(reason="a_d stored bf16"):
                        nc.vector.reduce_sum(
                            out=ad_sb[:, 2 * b:2 * b + 2].rearrange(
                                "p (a b) -> p a b", b=1),
                            in_=scr[:].rearrange("p (a c) -> p a c", c=D_H),
                            axis=mybir.AxisListType.X)
                    nc.scalar.dma_start(out=hwt_slab[b * P:(b + 1) * P, :], in_=tw[:])

            nc.gpsimd.collective_compute(
                "AllGather", ALU.bypass, replica_groups=rg,
                ins=[hwt_slab.opt()], outs=[hwt_full.opt()])

            # =============== Layer 2: GAT ===============
            with (
                tc.tile_pool(name="l2w", bufs=2) as wp,
                tc.tile_pool(name="l2g", bufs=2) as gp,
                tc.tile_pool(name="l2t", bufs=2) as tp2,
                tc.tile_pool(name="l2p", bufs=2, space="PSUM") as pp,
                tc.tile_pool(name="l2pc", bufs=2, space="PSUM") as ppc,
                tc.tile_pool(name="l2pa", bufs=2, space="PSUM") as ppa,
            ):
                NCHUNK = (T * P + 511) // 512
                for b in range(BPC):
                    g0 = gp.tile([P, Th * TBLW], F32, tag="g2a")
                    g1 = gp.tile([P, Th * TBLW], F32, tag="g2b")
                    for h, g in ((0, g0), (1, g1)):
                        src_ap = hwt_full[:] if h == 0 else hwt_full[HALF:, :]
                        ixt = gp.tile([P, NI // 16], I16, tag="ix2")
                        nc.sync.dma_start(out=ixt[:], in_=idx_d[b, h])
                        nc.gpsimd.dma_gather(
                            out_ap=g[:].rearrange("p (t w) -> p t w", w=TBLW),
                            in_ap=src_ap,
                            idxs_ap=ixt[:],
                            num_idxs=NI, num_idxs_reg=NI, elem_size=TBLW,
                            single_packet=False, queue_num=next_q())
                    mT = wp.tile([1, T * P], F32, tag="mT")
                    nc.sync.dma_start(out=mT[:], in_=metaT_d[b][None, :])
                    ad_ps = ppa.tile([P, 2 * T], F32, tag="adp")
                    for c in range(NCHUNK):
                        c0 = c * 512
                        c1 = min(T * P, c0 + 512)
                        cps = ppc.tile([P, 512], F32, tag="ck")
                        nc.tensor.matmul(out=cps[:, 0:c1 - c0], lhsT=onesr[:],
                                         rhs=mT[:, c0:c1], start=True, stop=True)
                        ohc = tp2.tile([P, 512], BF16, tag="ohT")
                        nc.vector.tensor_scalar(
                            out=ohc[:, 0:c1 - c0], in0=cps[:, 0:c1 - c0],
                            scalar1=iotac[:], scalar2=None, op0=ALU.is_equal)
                        for t in range(c0 // P, c1 // P):
                            nc.tensor.matmul(
                                out=ad_ps[:, 2 * t:2 * t + 2],
                                lhsT=ohc[:, t * P - c0:(t + 1) * P - c0],
                                rhs=ad_sb[:, 2 * b:2 * b + 2],
                                start=True, stop=True)
                    sc = wp.tile([P, 2 * T], F32, tag="sc")
                    for h, g in ((0, g0), (1, g1)):
                        nc.vector.tensor_tensor(
                            out=sc[:, h * 2 * Th:(h + 1) * 2 * Th].rearrange(
                                "p (t two) -> p t two", two=2),
                            in0=g[:].rearrange("p (t w) -> p t w", w=TBLW)[:, :, 256:258],
                            in1=ad_ps[:, h * 2 * Th:(h + 1) * 2 * Th].rearrange(
                                "p (t two) -> p t two", two=2),
                            op=ALU.add)
                    sc2 = wp.tile([P, 2 * T], F32, tag="sc2")
                    nc.vector.tensor_scalar(out=sc2[:], in0=sc[:], scalar1=NEG_SLOPE,
                                            scalar2=None, op0=ALU.mult)
                    nc.vector.tensor_tensor(out=sc[:], in0=sc[:], in1=sc2[:],
                                            op=ALU.max)
                    ex = wp.tile([P, 2 * T], F32, tag="ex")
                    nc.scalar.activation(out=ex[:], in_=sc[:], func=ACTF.Exp)
                    gat_ps = pp.tile([P, H * D_H + 2], F32, tag="pgat")
                    for t in range(T):
                        h, tr = divmod(t, Th)
                        g = g0 if h == 0 else g1
                        oh = wp.tile([P, P], F32, tag="oh2")
                        nc.vector.tensor_scalar(
                            out=oh[:], in0=iotar[:], scalar1=mcol(b, t),
                            scalar2=None, op0=ALU.is_equal)
                        mw = wp.tile([P, H * D_H + 2], F32, tag="mw")
                        nc.vector.tensor_scalar(
                            out=mw[:, 0:D_H], in0=g[:, tr * TBLW:tr * TBLW + D_H],
                            scalar1=ex[:, 2 * t:2 * t + 1], scalar2=None, op0=ALU.mult)
                        nc.vector.tensor_scalar(
                            out=mw[:, D_H:2 * D_H],
                            in0=g[:, tr * TBLW + D_H:tr * TBLW + 2 * D_H],
                            scalar1=ex[:, 2 * t + 1:2 * t + 2], scalar2=None,
                            op0=ALU.mult)
                        nc.vector.tensor_copy(out=mw[:, 2 * D_H:2 * D_H + 2],
                                              in_=ex[:, 2 * t:2 * t + 2])
                        nc.tensor.matmul(out=gat_ps[:], lhsT=oh[:], rhs=mw[:],
                                         start=(t == 0), stop=(t == T - 1))
                    s2 = wp.tile([P, 2], F32, tag="s2")
                    nc.vector.tensor_scalar(out=s2[:], in0=gat_ps[:, 256:258],
                                            scalar1=1e-30, scalar2=None, op0=ALU.add)
                    rec = wp.tile([P, 2], F32, tag="rec")
                    nc.vector.reciprocal(out=rec[:], in_=s2[:])
                    t0 = wp.tile([P, D_H], F32, tag="t0")
                    nc.vector.tensor_scalar(out=t0[:], in0=gat_ps[:, 0:D_H],
                                            scalar1=rec[:, 0:1], scalar2=None,
                                            op0=ALU.mult)
                    t1 = wp.tile([P, D_H], F32, tag="t1")
                    nc.vector.tensor_scalar(out=t1[:], in0=gat_ps[:, D_H:2 * D_H],
                                            scalar1=rec[:, 1:2], scalar2=None,
                                            op0=ALU.mult)
                    u2 = wp.tile([P, D_H], F32, tag="u2")
                    nc.vector.tensor_tensor(out=u2[:], in0=t0[:], in1=t1[:], op=ALU.add)
                    h2_blk = h2_sb[:, b * P:(b + 1) * P]
                    nc.scalar.activation(out=h2_blk, in_=u2[:], func=ACTF.Relu,
                                         scale=0.5)
                    nc.scalar.dma_start(out=h2_slab[b * P:(b + 1) * P, :], in_=h2_blk)

            nc.gpsimd.collective_compute(
                "AllGather", ALU.bypass, replica_groups=rg,
                ins=[h2_slab.opt()], outs=[h2_full.opt()])

            # =============== Layer 3: SAGE + output ===============
            with (
                tc.tile_pool(name="l3w", bufs=3) as wp,
                tc.tile_pool(name="l3p", bufs=2, space="PSUM") as pp,
                tc.tile_pool(name="l3pt", bufs=2, space="PSUM") as ppt,
                tc.tile_pool(name="l3po", bufs=1, space="PSUM") as ppo,
                tc.tile_pool(name="l3pl", bufs=1, space="PSUM") as ppl,
            ):
                for b in range(BPC):
                    g0 = wp.tile([P, Th * D_H], F32, tag="g3a")
                    g1 = wp.tile([P, Th * D_H], F32, tag="g3b")
                    for h, g in ((0, g0), (1, g1)):
                        src_ap = h2_full[:] if h == 0 else h2_full[HALF:, :]
                        ixt = wp.tile([P, NI // 16], I16, tag="ix3")
                        nc.sync.dma_start(out=ixt[:], in_=idx_d[b, h])
                        nc.gpsimd.dma_gather(
                            out_ap=g[:].rearrange("p (t w) -> p t w", w=D_H),
                            in_ap=src_ap,
                            idxs_ap=ixt[:],
                            num_idxs=NI, num_idxs_reg=NI, elem_size=D_H,
                            single_packet=False, queue_num=next_q())
                    psum = pp.tile([P, D_H], F32, tag="ps")
                    for t in range(T):
                        h, tr = divmod(t, Th)
                        g = g0 if h == 0 else g1
                        oh = wp.tile([P, P], F32, tag="oh3")
                        nc.vector.tensor_scalar(
                            out=oh[:], in0=iotar[:], scalar1=msage(b, t),
                            scalar2=None, op0=ALU.is_equal)
                        nc.tensor.matmul(out=psum[:], lhsT=oh[:],
                                         rhs=g[:, tr * D_H:(tr + 1) * D_H],
                                         start=(t == 0), stop=(t == T - 1))
                    recd = wp.tile([P, 1], F32, tag="recd")
                    nc.vector.reciprocal(out=recd[:],
                                         in_=degs_res[:, 2 * b + 1:2 * b + 2])
                    agg = wp.tile([P, D_H], F32, tag="agg")
                    nc.vector.tensor_scalar(out=agg[:], in0=psum[:], scalar1=recd[:],
                                            scalar2=None, op0=ALU.mult)
                    tps = ppt.tile([P, P], F32, tag="tr3")
                    nc.tensor.transpose(out=tps[:], in_=agg[:], identity=ident[:])
                    aggT = wp.tile([P, P], F32, tag="aggT")
                    nc.vector.tensor_copy(out=aggT[:], in_=tps[:])
                    tps2 = ppt.tile([P, P], F32, tag="tr3")
                    nc.tensor.transpose(out=tps2[:], in_=h2_sb[:, b * P:(b + 1) * P],
                                        identity=ident[:])
                    h2T = wp.tile([P, P], F32, tag="h2T")
                    nc.vector.tensor_copy(out=h2T[:], in_=tps2[:])
                    ops = ppo.tile([P, D_H], F32, tag="po")
                    nc.tensor.matmul(out=ops[:], lhsT=aggT[:], rhs=w_sl[:],
                                     start=True, stop=False)
                    nc.tensor.matmul(out=ops[:], lhsT=h2T[:], rhs=w_sr[:],
                                     start=False, stop=True)
                    h3 = h1_sb[:, b * P:(b + 1) * P]  # reuse h1 slab for h3
                    nc.scalar.activation(out=h3, in_=ops[:], func=ACTF.Relu)
                    tps3 = ppt.tile([P, P], F32, tag="tr3")
                    nc.tensor.transpose(out=tps3[:], in_=h3, identity=ident[:])
                    h3T = wp.tile([P, P], F32, tag="h3T")
                    nc.vector.tensor_copy(out=h3T[:], in_=tps3[:])
                    lg = ppl.tile([P, D_OUT], F32, tag="lg")
                    nc.tensor.matmul(out=lg[:], lhsT=h3T[:], rhs=w_out[:],
                                     start=True, stop=True)
                    m = wp.tile([P, 1], F32, tag="m")
                    nc.vector.reduce_max(out=m[:], in_=lg[:], axis=mybir.AxisListType.X)
                    tl = wp.tile([P, D_OUT], F32, tag="tl")
                    nc.vector.tensor_scalar(out=tl[:], in0=lg[:], scalar1=m[:],
                                            scalar2=None, op0=ALU.subtract)
                    epx = wp.tile([P, D_OUT], F32, tag="epx")
                    nc.scalar.activation(out=epx[:], in_=tl[:], func=ACTF.Exp)
                    sacc = wp.tile([P, 1], F32, tag="sacc")
                    nc.vector.reduce_sum(out=sacc[:], in_=epx[:],
                                         axis=mybir.AxisListType.X)
                    lse = wp.tile([P, 1], F32, tag="lse")
                    nc.scalar.activation(out=lse[:], in_=sacc[:], func=ACTF.Ln)
                    ob = wp.tile([P, D_OUT], F32, tag="ob")
                    nc.vector.tensor_scalar(out=ob[:], in0=tl[:], scalar1=lse[:],
                                            scalar2=None, op0=ALU.subtract)
                    nc.sync.dma_start(out=out_d[b * P:(b + 1) * P, :], in_=ob[:])

    nc.compile()
    return nc


# ----------------------------------------------------------------------------
# Entry point
# ----------------------------------------------------------------------------

def kernel(x, W_gcn, b_gcn, W_gat, att_src, att_dst, b_gat,
           W_sage_l, b_sage_l, W_sage_r, W_out, b_out, edge_index):
    x = np.asarray(x, np.float32)
    N = x.shape[0]
    for bb in (b_gcn, b_gat, b_sage_l, b_out):
        assert not np.any(np.asarray(bb)), "nonzero biases not wired in"
    pk = _pack(np.asarray(edge_index), N)
    NPAD, BPC = pk["NPAD"], pk["BPC"]

    x_perm = np.zeros((NPAD, D_IN), np.float32)
    x_perm[pk["perm"]] = x

    nc = _build_program(pk)

    att_s_b = np.tile(np.asarray(att_src, np.float32).reshape(1, H * D_H),
                      (P, 1)).copy()
    att_d_b = np.tile(np.asarray(att_dst, np.float32).reshape(1, H * D_H),
                      (P, 1)).copy()
    common = {
        "x_perm": x_perm,
        "w_gcn": np.ascontiguousarray(W_gcn, np.float32),
        "w_gat": np.ascontiguousarray(W_gat, np.float32),
        "att_s": att_s_b, "att_d": att_d_b,
        "w_sl": np.ascontiguousarray(W_sage_l, np.float32),
        "w_sr": np.ascontiguousarray(W_sage_r, np.float32),
        "w_out": np.ascontiguousarray(W_out, np.float32),
        "ident": np.eye(P, dtype=np.float32),
        "iotar": np.ascontiguousarray(
            np.tile(np.arange(P, dtype=np.float32)[None, :], (P, 1))),
        "iotac": np.ascontiguousarray(np.arange(P, dtype=np.float32)[:, None]),
        "onesr": np.ones((1, P), np.float32),
    }
    in_maps = []
    for c in range(NC):
        m = dict(common)
        m["idx"] = np.ascontiguousarray(pk["idx"][c * BPC:(c + 1) * BPC])
        m["meta"] = np.ascontiguousarray(pk["meta"][c * BPC:(c + 1) * BPC])
        m["metaT"] = np.ascontiguousarray(pk["metaT"][c * BPC:(c + 1) * BPC])
        m["degs"] = np.ascontiguousarray(pk["degs"][c * BPC:(c + 1) * BPC])
        in_maps.append(m)

    trace = bool(os.environ.get("GNN_KERNEL_TRACE"))
    if trace:
        _install_ntff_shim()
    res = run_bass_kernel_spmd(nc, in_maps, core_ids=list(range(NC)), trace=trace)
    if trace and res.exec_time_ns:
        print(f"HW exec time: {res.exec_time_ns} ns")

    out_all = np.concatenate([r["out"] for r in res.results], axis=0)
    return np.ascontiguousarray(out_all[pk["perm"]].astype(np.float32))


def _install_ntff_shim():
    import types
    try:
        from antenv import axon_hooks  # noqa: F401
        return
    except ImportError:
        pass
    import antenv
    mod = types.ModuleType("antenv.axon_hooks")
    mod._hook = None
    mod.set_axon_ntff_profile_hook = lambda h: setattr(mod, "_hook", h)
    mod.get_axon_ntff_profile_hook = lambda: mod._hook
    sys.modules["antenv.axon_hooks"] = mod
    antenv.axon_hooks = mod
    try:
        from trn_agent_boot.trn_boot import _ntff_profile_via_ctypes
        hook = _ntff_profile_via_ctypes("/opt/axon/libaxon_pjrt.so")
        if hook is not None:
            mod.set_axon_ntff_profile_hook(hook)
    except Exception:
        pass

